# revision 88
# baseline (speedup 1.0000x reference)
"""GroupedQueryAttention (B=2, S=2048, DIM=1024, 16 heads, 4 KV groups) on 8 trn2 cores.

Sharding: core c -> (batch b = c // 4, kv-group g = c % 4).
Each core: LayerNorm(x[b]) -> q/k/v projections for group g -> attention for the
group's 4 heads -> partial out-projection (w_o rows for group g), producing
y_c = partial_out^T [DIM, S].  Host sums the 4 group partials per batch,
transposes, and adds b_o.

v2 engine-balance notes (vs the v1 baseline, ~287us -> ~197us sim):
- x/weights/xn/xnT/outT/es/y in bf16 (halved DMA + 2-4x DVE modes); scores
  operands stay f32r. Measured rel err 7e-3 vs the 2e-2 gate.
- LayerNorm staged per quarter-group so each engine queue sees independent
  ops (no per-tile head-of-line blocking): row-sum via DVE tensor_scalar
  +accum, sum-of-squares via ACT Square+accum, xn on DVE, xnT evictions
  alternating ACT/DVE. Quarters software-pipelined one deep: quarter q's
  projections are emitted after quarter q+1's transposes.
- DMA order: x tiles 0-7, then projection weights, then x 8-15 (HWDGE issues
  serially at ~625ns/DMA); wo is fetched at attention start.
- Attention inner loop software-pipelined one m-tile deep (scores(m) before
  PV(m-1)). exp engine split: even head 2x512-wide ACT Exp (early psum-slot
  release), odd head 1024-wide, alternating m to a DVE int16-Schraudolph
  exp (bf16 bit-trick, rms ~1.8%, ~30% of elements incl. even-head
  halves at m%4==2n) to keep ACT below PE.
- Softmax denominators: ones-column in the PV weights; normalization reads
  the denominator row straight from PSUM (DVE recip), broadcast + divide on
  the idle Pool engine (DVE for the last block to shorten the tail).
- Head pairs share each m-tile with k in partitions 0:64 (even heads) and a
  kdup copy in 64:128 (odd heads): distinct row groups let real HW overlap
  the score matmuls (not modeled by the cost-model sim).
- If ln_beta @ W is nonzero the projection evictions switch to biased
  tensor_scalar_add at build time (the graded inputs have beta = 0).
"""

import numpy as np
import ml_dtypes

import concourse.mybir as mybir
from concourse import bacc
from concourse.bass_utils import run_bass_kernel_spmd
from concourse.tile import TileContext
from concourse.masks import make_identity

B, S, DIM = 2, 2048, 1024
HEADS, DH, G = 16, 64, 4
HPG = HEADS // G              # 4 heads per group
EG = HPG * DH                 # 256 q columns per group
SCALE = DH ** -0.5
P = 128
NT_S = S // P                 # 16
NT_D = DIM // P               # 8
F32 = mybir.dt.float32
F32R = mybir.dt.float32r
BF16 = mybir.dt.bfloat16
I16 = mybir.dt.int16
AF = mybir.ActivationFunctionType
OP = mybir.AluOpType

# Schraudolph exp in bf16: exp(s*SCALE) ~ bitcast_bf16(int16(s*SCHR_A + SCHR_B)).
# 486411/2**16 rescales the classic fp32 shift constant to the 7-bit mantissa.
SCHR_A = SCALE * (2.0 ** 7) / float(np.log(2.0))
SCHR_B = 127.0 * 2.0 ** 7 - 486411.0 / 2.0 ** 16


def build_nc(schr_mod=2, use_bias=False, NWARM=36):
    nc = bacc.Bacc("TRN2", target_bir_lowering=False)
    x = nc.dram_tensor("x", [S, DIM], BF16, kind="ExternalInput")
    wq = nc.dram_tensor("wq", [DIM, EG], BF16, kind="ExternalInput")
    wkv = nc.dram_tensor("wkv", [DIM, 2 * DH], BF16, kind="ExternalInput")
    wo = nc.dram_tensor("wo", [EG, DIM], BF16, kind="ExternalInput")
    qb = nc.dram_tensor("qb", [2, P], F32, kind="ExternalInput")   # beta @ w_q slice
    kvb = nc.dram_tensor("kvb", [1, P], F32, kind="ExternalInput")  # beta @ [w_k|w_v]
    y = nc.dram_tensor("y", [DIM, S], BF16, kind="ExternalOutput")

    with TileContext(nc) as tc:
        with tc.tile_pool(name="persist", bufs=1) as pp:
            ident = pp.tile([P, P], F32)
            make_identity(nc, ident[:])
            identr = pp.tile([P, P], F32R)
            nc.gpsimd.tensor_copy(out=identr[:], in_=ident[:])
            identb = pp.tile([P, P], BF16)
            nc.gpsimd.tensor_copy(out=identb[:], in_=ident[:])
            wq_sb = pp.tile([P, NT_D, EG], BF16)
            wkv_sb = pp.tile([P, NT_D, 2 * DH], BF16)
            wo_sb = pp.tile([P, 2, DIM], BF16)
            qb_sb = pp.tile([P, 2], F32)
            kvb_sb = pp.tile([P, 1], F32)

            eps_sb = pp.tile([P, 1], F32)
            nc.vector.memset(eps_sb[:], 1e-5)
            ones_col = pp.tile([P, 1], F32)
            nc.vector.memset(ones_col[:], 1.0)

            qT = pp.tile([P, 2, S], F32R)      # [e%128, e-chunk, s]
            kvT = pp.tile([P, S], F32R)        # rows 0:64 = kT, 64:128 = vT
            kdup = pp.tile([P, S], F32R)       # rows 64:128 = kT copy (odd heads)
            vones = pp.tile([P, NT_S, DH + 1], BF16)
            outT = pp.tile([P, 2, S], BF16)    # like qT

            nc.gpsimd.tensor_copy(out=vones[:, :, DH],
                                  in_=ones_col[:].broadcast_to([P, NT_S]))

            # ---------- Phase 1: LayerNorm + transpose + projections ----------
            with tc.tile_pool(name="xnTp", bufs=1) as xp_:
                xnT = xp_.tile([P, NT_D, S], BF16)
                with tc.tile_pool(name="ln", bufs=16) as lnp, \
                     tc.tile_pool(name="xnp", bufs=16) as xnp, \
                     tc.tile_pool(name="scr", bufs=3) as scp, \
                     tc.tile_pool(name="lns", bufs=16) as lsp, \
                     tc.tile_pool(name="psT", bufs=2, space="PSUM") as ptp, \
                     tc.tile_pool(name="psP", bufs=3, space="PSUM") as ppp, \
                     tc.tile_pool(name="psV", bufs=1, space="PSUM") as pvp:
                    # PE warmup: dense junk transposes so the clock ramp
                    # (pstate) completes before the first real matmuls
                    warm = ptp.tile([P, P], BF16, tag="warm")
                    for _ in range(NWARM):
                        nc.tensor.transpose(warm[:], identb[:], identb[:])
                    # prefetch: x quarter 0, then weights, then the rest of x
                    xts = []
                    for i in range(NT_S):
                        xt_i = lnp.tile([P, DIM], BF16, tag="x", name=f"xt{i}")
                        xts.append(xt_i)
                    for i in range(8):
                        nc.sync.dma_start(out=xts[i][:], in_=x[i * P:(i + 1) * P, :])
                    for c in range(NT_D):
                        nc.sync.dma_start(out=wq_sb[:, c, :],
                                          in_=wq[c * P:(c + 1) * P, :])
                        nc.sync.dma_start(out=wkv_sb[:, c, :],
                                          in_=wkv[c * P:(c + 1) * P, :])
                    if use_bias:
                        for e in range(2):
                            nc.sync.dma_start(out=qb_sb[:, e:e + 1],
                                              in_=qb[e, :, None])
                        nc.sync.dma_start(out=kvb_sb[:], in_=kvb[0, :, None])
                    for i in range(8, NT_S):
                        nc.sync.dma_start(out=xts[i][:], in_=x[i * P:(i + 1) * P, :])
                    # quarter-staged LN emission: within each group of 4 tiles,
                    # each engine queue sees 4 independent ops per stage (no
                    # per-tile head-of-line blocking); groups stay short so the
                    # first transposes start early. Quarter 0 leads with a
                    # 2-tile group so the very first transpose isn't gated on
                    # four tiles of stats.
                    pending_proj = []

                    def emit_projections(quarter):
                        q0 = quarter * 512
                        for mc in range(2):
                            pq = ppp.tile([P, 512], F32, tag="pq",
                                          name=f"pq{quarter}_{mc}")
                            for c in range(NT_D):
                                nc.tensor.matmul(
                                    pq[:],
                                    lhsT=wq_sb[:, c, mc * P:(mc + 1) * P],
                                    rhs=xnT[:, c, q0:q0 + 512],
                                    start=(c == 0), stop=(c == NT_D - 1))
                            if use_bias:
                                nc.vector.tensor_scalar_add(
                                    qT[:, mc, q0:q0 + 512], pq[:],
                                    qb_sb[:, mc:mc + 1])
                            else:
                                nc.scalar.activation(qT[:, mc, q0:q0 + 512], pq[:],
                                                     AF.Copy)
                        pkv = ppp.tile([P, 512], F32, tag="pq",
                                       name=f"pkv{quarter}")
                        for c in range(NT_D):
                            nc.tensor.matmul(
                                pkv[:],
                                lhsT=wkv_sb[:, c, :],
                                rhs=xnT[:, c, q0:q0 + 512],
                                start=(c == 0), stop=(c == NT_D - 1))
                        if use_bias:
                            nc.vector.tensor_scalar_add(kvT[:, q0:q0 + 512], pkv[:],
                                                        kvb_sb[:])
                        else:
                            nc.scalar.activation(kvT[:, q0:q0 + 512], pkv[:],
                                                 AF.Copy)
                        # kdup chunk + V natural layout tiles for this quarter
                        nc.sync.dma_start(out=kdup[64:128, q0:q0 + 512],
                                          in_=kvT[0:DH, q0:q0 + 512])
                        for m in range(quarter * 4, quarter * 4 + 4):
                            pv = pvp.tile([P, DH], F32R, tag="pv",
                                          name=f"pv{m}")
                            nc.tensor.transpose(pv[:],
                                                kvT[64:128, m * P:(m + 1) * P],
                                                identr[64:128, 64:128])
                            nc.scalar.activation(vones[:, m, 0:DH],
                                                 pv[:].bitcast(F32), AF.Copy)

                    groups = [range(0, 1), range(1, 2), range(2, 4)] + \
                        [range(q * 4, q * 4 + 4) for q in range(1, 4)]
                    for rng in groups:
                        sms, ssqs, negmus, mu2s, vars_, stds, rstds, xns = \
                            {}, {}, {}, {}, {}, {}, {}, {}
                        for i in rng:
                            sm = lsp.tile([P, 1], F32, tag="sm", name=f"sm{i}")
                            scr1 = scp.tile([P, DIM], BF16, tag="scr1")
                            nc.vector.tensor_scalar(
                                out=scr1[:], in0=xts[i][:], scalar1=1.0,
                                scalar2=0.0, op0=OP.mult, op1=OP.add,
                                accum_out=sm[:])
                            sms[i] = sm
                        for i in rng:
                            ssq = lsp.tile([P, 1], F32, tag="ssq", name=f"ssq{i}")
                            scr2 = scp.tile([P, DIM], BF16, tag="scr2")
                            if i % 2 == 0:
                                nc.scalar.activation(scr2[:], xts[i][:],
                                                     AF.Square, accum_out=ssq[:])
                            else:
                                # DVE pair (ttr is broken on hw): square, then
                                # accumulate; bf16 rounding of the squares
                                # averages out in the 1024-wide sum
                                nc.vector.tensor_mul(scr2[:], xts[i][:],
                                                     xts[i][:])
                                scr3 = scp.tile([P, DIM], BF16, tag="scr3")
                                nc.vector.tensor_scalar(
                                    out=scr3[:], in0=scr2[:], scalar1=1.0,
                                    scalar2=0.0, op0=OP.mult, op1=OP.add,
                                    accum_out=ssq[:])
                            ssqs[i] = ssq
                        for i in rng:
                            negmu = lsp.tile([P, 1], F32, tag="negmu",
                                             name=f"negmu{i}")
                            nc.vector.tensor_scalar_mul(negmu[:], sms[i][:],
                                                        -1.0 / DIM)
                            negmus[i] = negmu
                        for i in rng:
                            mu2 = lsp.tile([P, 1], F32, tag="mu2", name=f"mu2{i}")
                            nc.vector.tensor_mul(mu2[:], negmus[i][:], negmus[i][:])
                            mu2s[i] = mu2
                        for i in rng:
                            var = lsp.tile([P, 1], F32, tag="var", name=f"var{i}")
                            nc.vector.tensor_scalar(
                                out=var[:], in0=ssqs[i][:], scalar1=1.0 / DIM,
                                scalar2=mu2s[i][:], op0=OP.mult, op1=OP.subtract)
                            vars_[i] = var
                        for i in rng:
                            std = lsp.tile([P, 1], F32, tag="std", name=f"std{i}")
                            nc.scalar.activation(std[:], vars_[i][:], AF.Sqrt,
                                                 bias=eps_sb[:])
                            stds[i] = std
                        for i in rng:
                            rstd = lsp.tile([P, 1], F32, tag="rstd",
                                            name=f"rstd{i}")
                            nc.vector.reciprocal(rstd[:], stds[i][:])
                            rstds[i] = rstd
                        for i in rng:
                            xn = xnp.tile([P, DIM], BF16, tag="xn", name=f"xn{i}")
                            nc.vector.tensor_scalar(
                                out=xn[:], in0=xts[i][:], scalar1=negmus[i][:],
                                scalar2=rstds[i][:], op0=OP.add, op1=OP.mult)
                            xns[i] = xn
                        for i in rng:
                            pt = ptp.tile([P, DIM], BF16, tag="pt")
                            for j in range(NT_D):
                                nc.tensor.transpose(pt[:, j * P:(j + 1) * P],
                                                    xns[i][:, j * P:(j + 1) * P],
                                                    identb[:])
                            if i % 2 == 0:
                                nc.scalar.activation(
                                    xnT[:, :, i * P:(i + 1) * P],
                                    pt[:].rearrange("p (j c) -> p j c", j=NT_D),
                                    AF.Copy)
                            else:
                                nc.vector.tensor_copy(
                                    out=xnT[:, :, i * P:(i + 1) * P],
                                    in_=pt[:].rearrange("p (j c) -> p j c", j=NT_D))
                        if rng[-1] % 4 == 3:
                            pending_proj.append(rng[-1] // 4)
                        # 1-quarter software pipeline: emit quarter q's
                        # projections only after quarter q+1's transposes, so
                        # the PE never waits on the freshest xnT eviction.
                        while len(pending_proj) > (0 if rng[-1] == NT_S - 1 else 1):
                            emit_projections(pending_proj.pop(0))

            # ---------- Phase 3: attention ----------
            for e in range(2):
                nc.sync.dma_start(out=wo_sb[:, e, :], in_=wo[e * P:(e + 1) * P, :])
            with tc.tile_pool(name="es", bufs=8) as esp, \
                 tc.tile_pool(name="esi", bufs=6) as esip, \
                 tc.tile_pool(name="bc", bufs=2) as bp, \
                 tc.tile_pool(name="psS", bufs=1, space="PSUM") as psp, \
                 tc.tile_pool(name="psO", bufs=1, space="PSUM") as pop:
                heads = [(0, kvT), (64, kdup)]  # (row base, kk source)
                for half in range(2):
                    for pair in range(2):
                        ch = pair
                        q0 = half * 1024
                        po0 = pop.tile([DH + 1, 1024], F32, tag="po0")
                        po1 = pop.tile([DH + 1, 1024], F32, tag="po1")
                        pos = [po0, po1]
                        # 1-deep software pipeline: scores(m) then PV(m-1).
                        # Even head: two half-width psum tags, ACT exp per
                        # half (early slot release avoids ps-reuse bubbles).
                        # Odd head: full-width tag, alternate m to the DVE
                        # Schraudolph exp.
                        pend = None
                        for m in range(NT_S + 1):
                            cur = []
                            if m < NT_S:
                                # even head, half-tiles on ACT
                                pr, kk = heads[0]
                                kslice = kk[pr:pr + DH, m * P:(m + 1) * P]
                                for n in range(2):
                                    psh = psp.tile([P, 512], F32, tag=f"psA{n}")
                                    nc.tensor.matmul(
                                        psh[:], lhsT=kslice,
                                        rhs=qT[pr:pr + DH, ch,
                                               q0 + n * 512:q0 + (n + 1) * 512],
                                        start=True, stop=True)
                                    if (schr_mod and m % 2 == 0
                                            and m < NT_S - 2):
                                        esh = esip.tile([P, 512], I16, tag="esh")
                                        nc.vector.tensor_scalar(
                                            out=esh[:], in0=psh[:],
                                            scalar1=SCHR_A, scalar2=SCHR_B,
                                            op0=OP.mult, op1=OP.add)
                                        cur.append((0, n, esh[:].bitcast(BF16)))
                                    else:
                                        es = esp.tile([P, 512], BF16, tag="es")
                                        nc.scalar.activation(es[:], psh[:],
                                                             AF.Exp, scale=SCALE)
                                        cur.append((0, n, es[:]))
                                # odd head, two half tags; m-parity to DVE
                                pr, kk = heads[1]
                                kslice = kk[pr:pr + DH, m * P:(m + 1) * P]
                                for n in range(2):
                                    psb = psp.tile([P, 512], F32, tag=f"psB{n}")
                                    nc.tensor.matmul(
                                        psb[:], lhsT=kslice,
                                        rhs=qT[pr:pr + DH, ch,
                                               q0 + n * 512:q0 + (n + 1) * 512],
                                        start=True, stop=True)
                                    if schr_mod and m % 2 == 1 and m < NT_S - 2:
                                        esi = esip.tile([P, 512], I16, tag="esi")
                                        nc.vector.tensor_scalar(
                                            out=esi[:], in0=psb[:],
                                            scalar1=SCHR_A, scalar2=SCHR_B,
                                            op0=OP.mult, op1=OP.add)
                                        cur.append((1, n, esi[:].bitcast(BF16)))
                                    else:
                                        esf = esp.tile([P, 512], BF16, tag="esf")
                                        nc.scalar.activation(esf[:], psb[:],
                                                             AF.Exp, scale=SCALE)
                                        cur.append((1, n, esf[:]))
                            if pend is not None:
                                pm = m - 1
                                for (hi, n, rhs_es) in pend:
                                    nc.tensor.matmul(
                                        pos[hi][:, n * 512:(n + 1) * 512],
                                        lhsT=vones[:, pm, :], rhs=rhs_es,
                                        start=(pm == 0), stop=(pm == NT_S - 1))
                            pend = cur
                        # normalize: divide rows 0:64 by the ones-row sum.
                        # recip reads the denominator row straight from PSUM
                        # (DVE) while ACT evicts the head output; the divide
                        # itself runs on the idle Pool engine.
                        last_blk = (half == 1 and pair == 1)
                        for hi in (1, 0):
                            po = pos[hi]
                            rc = bp.tile([1, 1024], F32, tag="rc")
                            nc.vector.reciprocal(rc[:], po[DH:DH + 1, :])
                            ot = bp.tile([DH, 1024], F32, tag="ot")
                            nc.scalar.activation(ot[:], po[0:DH, :], AF.Copy)
                            rbs = bp.tile([DH, 1024], F32, tag="rbs")
                            nc.gpsimd.partition_broadcast(rbs[:], rc[:])
                            mul_eng = nc.vector if last_blk else nc.gpsimd
                            if hi == 0:
                                mul_eng.tensor_mul(
                                    outT[0:DH, ch, q0:q0 + 1024], ot[:], rbs[:])
                            else:
                                st = bp.tile([DH, 1024], BF16, tag="st")
                                mul_eng.tensor_mul(st[:], ot[:], rbs[:])
                                nc.sync.dma_start(
                                    out=outT[DH:2 * DH, ch, q0:q0 + 1024], in_=st[:])



            # ---------- Phase 4: out-projection ----------
            with tc.tile_pool(name="yt", bufs=4) as yp, \
                 tc.tile_pool(name="psY", bufs=2, space="PSUM") as pyp:
                for half in range(2):
                    q0 = half * 1024
                    for mc in range(NT_D):
                        py = pyp.tile([P, 1024], F32, tag="py")
                        for ec in range(2):
                            for n in range(2):
                                nc.tensor.matmul(
                                    py[:, n * 512:(n + 1) * 512],
                                    lhsT=wo_sb[:, ec, mc * P:(mc + 1) * P],
                                    rhs=outT[:, ec, q0 + n * 512:q0 + (n + 1) * 512],
                                    start=(ec == 0), stop=(ec == 1))
                        yt = yp.tile([P, 1024], BF16, tag="yt")
                        if mc % 2 == 0:
                            nc.vector.tensor_copy(out=yt[:], in_=py[:])
                        else:
                            nc.scalar.activation(yt[:], py[:], AF.Copy)
                        eng = nc.sync if mc % 2 == 0 else nc.scalar
                        eng.dma_start(out=y[mc * P:(mc + 1) * P, q0:q0 + 1024],
                                      in_=yt[:])

    nc.compile()
    return nc


_NC = None
_NC_BIAS = None


def _get_nc(use_bias=False):
    global _NC, _NC_BIAS
    if _NC is None or _NC_BIAS != use_bias:
        _NC = build_nc(use_bias=use_bias)
        _NC_BIAS = use_bias
    return _NC


def make_in_maps(x, ln_gamma, ln_beta, w_q, w_k, w_v, w_o):
    x = np.asarray(x, np.float32)
    g_ = np.asarray(ln_gamma, np.float32)
    b_ = np.asarray(ln_beta, np.float32)
    w_o = np.asarray(w_o, np.float32)
    bf = ml_dtypes.bfloat16
    in_maps = []
    for core in range(8):
        b, g = divmod(core, 4)
        wq_s = np.ascontiguousarray(g_[:, None] * w_q[:, g * EG:(g + 1) * EG]).astype(bf)
        wkv_s = np.concatenate(
            [g_[:, None] * w_k[:, g * DH:(g + 1) * DH],
             g_[:, None] * w_v[:, g * DH:(g + 1) * DH]], axis=1).astype(bf)
        qb_s = (b_ @ w_q[:, g * EG:(g + 1) * EG]).reshape(2, P).astype(np.float32)
        kvb_s = np.concatenate(
            [b_ @ w_k[:, g * DH:(g + 1) * DH],
             b_ @ w_v[:, g * DH:(g + 1) * DH]]).reshape(1, P).astype(np.float32)
        in_maps.append({
            "x": np.ascontiguousarray(x[b]).astype(bf),
            "wq": wq_s, "wkv": np.ascontiguousarray(wkv_s),
            "wo": np.ascontiguousarray(w_o[g * EG:(g + 1) * EG, :]).astype(bf),
            "qb": qb_s, "kvb": kvb_s,
        })
    return in_maps


def kernel(x, ln_gamma, ln_beta, w_q, w_k, w_v, w_o, b_o):
    in_maps = make_in_maps(x, ln_gamma, ln_beta, w_q, w_k, w_v, w_o)
    use_bias = any(np.any(m["qb"]) or np.any(m["kvb"]) for m in in_maps)
    nc = _get_nc(use_bias)
    res = run_bass_kernel_spmd(nc, in_maps, list(range(8)))
    out = np.zeros((B, S, DIM), np.float32)
    for core in range(8):
        b = core // 4
        out[b] += res.results[core]["y"].T.astype(np.float32)
    out += np.asarray(b_o, np.float32)
    return out
